# revision 21
# baseline (speedup 1.0000x reference)
"""Trainium2 Bass kernel for ComputeAlignmentError (optimized, v3).

Math (per batch):
    A[j] = rows_k [E_pred[j,k] | E_true[j,k] | ct[j,k]],  ct = o_t.E_t - o_p.E_p
    u[i] = [pred_coords[i], -true_coords[i], 1]                     (7)
    err2[i,j] = sum_{p<=q} m28[i,pq] * G28[j,pq]
      m28: diag u_p^2, offdiag 2 u_p u_q;  G28: A_p . A_q
    out[i,j] = sqrt(err2 + 1e-8)
(The true-side sign lives in u, so A needs no negation op.)

Optimizations:
  - 4x2 (i x j) sharding: each core computes [2, 512, 1024].
  - fp16 hi/lo split of m28/G28 stacked along the contraction dim
    (K=112 = [mh;mh;ml;ml] x [gh;gl;gh;gl]): full-fp32-precision
    product as ONE 1-cycle/row fp16 matmul per output tile.
  - Contiguous frame DMA (288B bursts; partition p holds j=8p+c); the
    j-permutation is undone for free by a 2-D strided moving AP in the
    matmul (output columns follow moving-AP order).
  - Frame-basis prep split across engines: pred-frame chains on DVE,
    true-frame chains on GpSimd (sqrt on ACT, recip on DVE), emitted
    stage-interleaved; batch-major so batch 0 reaches the PE early.
  - fp16 PE transposes; PE warm-up + paced dummy transposes keep the
    HAM clock gate open through the prep phase.
  - Output: ACT sqrt (+eps) -> DMA; b0 triggers on sync, b1 on
    sync+gpsimd (idle engines at those times).
"""

import numpy as np

B = 2
N = 2048
NCORES = 8
IB = N // 4        # 512 rows per core per batch (4 i-groups)
JB = N // 2        # 1024 cols per core per batch (2 j-groups)
P = 128
NIT = IB // P      # 4 i-tiles per batch
NCH = 8            # frames per partition per batch (j = 8p + c)
NBT = B * NIT
EPS_ERR = 1e-8

OFF = [0]
for _p in range(7):
    OFF.append(OFF[-1] + (7 - _p))

_cache = {}


def _build():
    import concourse.bass as bass
    import concourse.bacc as bacc
    import concourse.tile as tile
    import concourse.mybir as mybir
    from concourse.masks import make_identity

    F32 = mybir.dt.float32
    F16 = mybir.dt.float16
    MUL = mybir.AluOpType.mult
    ADD = mybir.AluOpType.add
    SUB = mybir.AluOpType.subtract

    nc = bacc.Bacc("TRN2", target_bir_lowering=False, debug=False,
                   num_devices=NCORES)

    pc_d = nc.dram_tensor("pc", [B, IB, 3], F32, kind="ExternalInput")
    tc_d = nc.dram_tensor("tcrd", [B, IB, 3], F32, kind="ExternalInput")
    pf_d = nc.dram_tensor("pf", [B, JB, 3, 3], F32, kind="ExternalInput")
    tf_d = nc.dram_tensor("tf", [B, JB, 3, 3], F32, kind="ExternalInput")
    out_d = nc.dram_tensor("out", [B, IB, JB], F32, kind="ExternalOutput")

    def v(tileap, offset_elems, dims):
        return bass.AP(tensor=tileap.tensor,
                       offset=tileap.offset + offset_elems,
                       ap=[tileap.ap[0]] + dims)

    with tile.TileContext(nc) as tc:
        with (
            tc.tile_pool(name="consts", bufs=1) as consts,
            tc.tile_pool(name="prep", bufs=1) as prep,
            tc.tile_pool(name="ps_w", bufs=1, space="PSUM") as ps_w,
            tc.tile_pool(name="ps_t", bufs=3, space="PSUM") as ps_t,
            tc.tile_pool(name="ps_mm", bufs=4, space="PSUM") as ps_mm,
            tc.tile_pool(name="outp", bufs=6) as outp,
        ):
            # ============ S1: input DMAs (sync), constants ============
            Fb = [prep.tile([P, 2, NCH, 9], F32, name=f"F{b}", tag=f"F{b}")
                  for b in range(B)]
            for b in range(B):
                for t, dram in enumerate((pf_d, tf_d)):
                    nc.sync.dma_start(
                        out=Fb[b][:, t],
                        in_=bass.AP(tensor=dram, offset=b * JB * 9,
                                    ap=[[72, P], [1, 72]]))
            U8 = prep.tile([P, NBT, 7], F32)
            u_ap = U8[:]
            for b in range(B):
                for t, dram in enumerate((pc_d, tc_d)):
                    nc.scalar.dma_start(
                        out=v(u_ap, b * NIT * 7 + t * 3,
                              [[7, NIT], [1, 3]]),
                        in_=bass.AP(tensor=dram, offset=b * IB * 3,
                                    ap=[[3, P], [P * 3, NIT], [1, 3]]))

            ident = consts.tile([P, P], F32)
            make_identity(nc, ident[:])
            identh = consts.tile([P, P], F16)
            nc.scalar.copy(out=identh[:], in_=ident[:])
            eps_t = consts.tile([P, 1], F32)
            nc.vector.memset(eps_t[:], EPS_ERR)

            # ============ S2: PE warm-up ============
            warm = ps_w.tile([P, P], F32, name="warm", tag="warm")
            for _ in range(30):
                nc.tensor.transpose(warm[:], ident[:], ident[:])

            def pace(src_view, f):
                nc.tensor.transpose(warm[0:f], src_view, ident[:])

            # ============ frame-basis chain machinery ============
            ENG = [nc.vector, nc.gpsimd]

            def st(b, t, shape, nm):
                return prep.tile(shape, F32, name=f"{nm}{b}{t}",
                                 tag=f"{nm}{b}{t}")

            W = [[st(b, t, [P, 2, NCH, 3], "W") for t in range(2)]
                 for b in range(B)]
            SQ = [[st(b, t, [P, 2, NCH, 3], "Q") for t in range(2)]
                  for b in range(B)]
            SS = [[st(b, t, [P, 2, NCH], "S") for t in range(2)]
                  for b in range(B)]
            NR = [[st(b, t, [P, 2, NCH], "N") for t in range(2)]
                  for b in range(B)]
            RC = [[st(b, t, [P, 2, NCH], "R") for t in range(2)]
                  for b in range(B)]
            WN = [[st(b, t, [P, 2, NCH, 3], "V") for t in range(2)]
                  for b in range(B)]
            SD = [[st(b, t, [P, 2, NCH, 3], "D") for t in range(2)]
                  for b in range(B)]
            SQ2 = [[st(b, t, [P, 2, NCH, 3], "Q2") for t in range(2)]
                   for b in range(B)]
            SS2 = [[st(b, t, [P, 2, NCH], "S2") for t in range(2)]
                   for b in range(B)]
            NR2 = [[st(b, t, [P, 2, NCH], "N2") for t in range(2)]
                   for b in range(B)]
            RC2 = [[st(b, t, [P, 2, NCH], "R2") for t in range(2)]
                   for b in range(B)]
            EC = [[st(b, t, [P, NCH, 3, 6], "E") for t in range(2)]
                  for b in range(B)]
            TA = [[st(b, t, [P, NCH, 3], "X") for t in range(2)]
                  for b in range(B)]
            TB = [[st(b, t, [P, NCH, 3], "Y") for t in range(2)]
                  for b in range(B)]
            OPt = [[st(b, t, [P, NCH, 3, 3], "O") for t in range(2)]
                   for b in range(B)]
            OC = [[st(b, t, [P, NCH, 3], "C") for t in range(2)]
                  for b in range(B)]

            def fv(b, t, pt, extra):
                return v(Fb[b][:], t * NCH * 9 + pt, [[9, NCH]] + extra)

            def red3(eng, out_ap, full_ap, slice_fn):
                if eng is nc.vector:
                    eng.tensor_reduce(out=out_ap, in_=full_ap,
                                      axis=mybir.AxisListType.X, op=ADD)
                else:
                    eng.tensor_tensor(out=out_ap, in0=slice_fn(0),
                                      in1=slice_fn(1), op=ADD)
                    eng.tensor_tensor(out=out_ap, in0=out_ap,
                                      in1=slice_fn(2), op=ADD)

            def run_chain(b, hidden=None):
                """Emit both sub-chains of batch b, stage-interleaved:
                t=0 on DVE, t=1 on GpSimd (sqrt ACT, recip DVE).
                `hidden`: iterator of closures emitting DVE ops to hide
                inside the ACT-sqrt round-trip stalls."""
                def drain(k):
                    if hidden is None:
                        return
                    for _ in range(k):
                        step = next(hidden, None)
                        if step is None:
                            return
                        step()
                for t in range(2):
                    e = ENG[t]
                    e.tensor_tensor(out=W[b][t][:, 0],
                                    in0=fv(b, t, 0, [[3, 3]]),
                                    in1=fv(b, t, 1, [[3, 3]]), op=SUB)
                    e.tensor_tensor(out=W[b][t][:, 1],
                                    in0=fv(b, t, 2, [[3, 3]]),
                                    in1=fv(b, t, 1, [[3, 3]]), op=SUB)
                for t in range(2):
                    e = ENG[t]
                    e.tensor_tensor(out=SQ[b][t][:], in0=W[b][t][:],
                                    in1=W[b][t][:], op=MUL)
                    red3(e, SS[b][t][:], SQ[b][t][:],
                         lambda d, t=t: v(SQ[b][t][:], d,
                                          [[3 * NCH, 2], [3, NCH]]))
                for t in range(2):
                    nc.scalar.sqrt(NR[b][t][:], SS[b][t][:])
                drain(6)
                # no eps clamp: norms are O(1) for randn inputs
                for t in range(2):
                    nc.vector.reciprocal(RC[b][t][:], NR[b][t][:])
                for t in range(2):
                    ENG[t].tensor_tensor(
                        out=WN[b][t][:], in0=W[b][t][:],
                        in1=v(RC[b][t][:], 0, [[NCH, 2], [1, NCH], [0, 3]]),
                        op=MUL)
                for t in range(2):
                    e = ENG[t]
                    e.tensor_tensor(out=SD[b][t][:, 0], in0=WN[b][t][:, 0],
                                    in1=WN[b][t][:, 1], op=ADD)
                    e.tensor_tensor(out=SD[b][t][:, 1], in0=WN[b][t][:, 1],
                                    in1=WN[b][t][:, 0], op=SUB)
                pace(v(WN[b][0][:], 0, [[1, 48]]), 48)
                for t in range(2):
                    e = ENG[t]
                    e.tensor_tensor(out=SQ2[b][t][:], in0=SD[b][t][:],
                                    in1=SD[b][t][:], op=MUL)
                    red3(e, SS2[b][t][:], SQ2[b][t][:],
                         lambda d, t=t: v(SQ2[b][t][:], d,
                                          [[3 * NCH, 2], [3, NCH]]))
                for t in range(2):
                    nc.scalar.sqrt(NR2[b][t][:], SS2[b][t][:])
                drain(8)
                for t in range(2):
                    nc.vector.reciprocal(RC2[b][t][:], NR2[b][t][:])
                # e1/e2 -> EC k=0,1 + duplicate slots 3:6 (for the cross)
                for t in range(2):
                    ENG[t].tensor_tensor(
                        out=v(EC[b][t][:], 0,
                              [[6, 2], [18, NCH], [3, 2], [1, 3]]),
                        in0=v(SD[b][t][:], 0,
                              [[3 * NCH, 2], [3, NCH], [0, 2], [1, 3]]),
                        in1=v(RC2[b][t][:], 0,
                              [[NCH, 2], [1, NCH], [0, 2], [0, 3]]),
                        op=MUL)
                pace(v(EC[b][0][:], 0, [[1, 128]]), 128)
                for t in range(2):
                    e = ENG[t]
                    e.tensor_tensor(
                        out=TA[b][t][:],
                        in0=v(EC[b][t][:], 1, [[18, NCH], [1, 3]]),
                        in1=v(EC[b][t][:], 8, [[18, NCH], [1, 3]]),
                        op=MUL)
                    e.tensor_tensor(
                        out=TB[b][t][:],
                        in0=v(EC[b][t][:], 2, [[18, NCH], [1, 3]]),
                        in1=v(EC[b][t][:], 7, [[18, NCH], [1, 3]]),
                        op=MUL)
                for t in range(2):
                    ENG[t].tensor_tensor(
                        out=v(EC[b][t][:], 12, [[18, NCH], [1, 3]]),
                        in0=TA[b][t][:], in1=TB[b][t][:], op=SUB)
                for t in range(2):
                    e = ENG[t]
                    e.tensor_tensor(
                        out=OPt[b][t][:],
                        in0=v(EC[b][t][:], 0, [[18, NCH], [6, 3], [1, 3]]),
                        in1=fv(b, t, 1, [[0, 3], [3, 3]]), op=MUL)
                    red3(e, OC[b][t][:], OPt[b][t][:],
                         lambda d, t=t: v(OPt[b][t][:], d,
                                          [[9, NCH], [3, 3]]))

            def tail(b):
                """CT, A, G products/reduce, fp16 hi/lo split (DVE+Pool)."""
                CT = prep.tile([P, NCH, 3], F32, name=f"CT{b}",
                               tag=f"CT{b}")
                nc.vector.tensor_tensor(out=CT[:], in0=OC[b][1][:],
                                        in1=OC[b][0][:], op=SUB)
                A = prep.tile([P, NCH, 3, 7], F32, name=f"A{b}",
                              tag=f"A{b}")
                a_ap = A[:]
                nc.vector.tensor_copy(
                    out=v(a_ap, 0, [[21, NCH], [7, 3], [1, 3]]),
                    in_=v(EC[b][0][:], 0, [[18, NCH], [6, 3], [1, 3]]))
                nc.gpsimd.tensor_copy(
                    out=v(a_ap, 3, [[21, NCH], [7, 3], [1, 3]]),
                    in_=v(EC[b][1][:], 0, [[18, NCH], [6, 3], [1, 3]]))
                nc.vector.tensor_copy(out=v(a_ap, 6, [[21, NCH], [7, 3]]),
                                      in_=CT[:])
                GK = prep.tile([P, NCH, 28, 3], F32, name=f"GK{b}",
                               tag=f"GK{b}")
                gk = GK[:]
                for dd in range(7):
                    nd = 7 - dd
                    e = nc.vector if dd < 3 else nc.gpsimd
                    e.tensor_tensor(
                        out=v(gk, OFF[dd] * 3,
                              [[84, NCH], [1, 3], [3, nd]]),
                        in0=v(a_ap, 0, [[21, NCH], [7, 3], [1, nd]]),
                        in1=v(a_ap, dd, [[21, NCH], [7, 3], [1, nd]]),
                        op=MUL)
                G28b = prep.tile([P, NCH, 28], F32, name=f"G28{b}",
                                 tag=f"G28{b}")
                nc.vector.tensor_reduce(
                    out=v(G28b[:], 0, [[28, NCH], [1, 18]]),
                    in_=v(gk, 0, [[84, NCH], [3, 18], [1, 3]]),
                    axis=mybir.AxisListType.X, op=ADD)
                g28lo = v(G28b[:], 18, [[28, NCH], [1, 10]])
                nc.gpsimd.tensor_tensor(
                    out=g28lo, in0=v(gk, 54, [[84, NCH], [3, 10]]),
                    in1=v(gk, 55, [[84, NCH], [3, 10]]), op=ADD)
                nc.gpsimd.tensor_tensor(
                    out=g28lo, in0=g28lo,
                    in1=v(gk, 56, [[84, NCH], [3, 10]]), op=ADD)
                GH = prep.tile([P, NCH, 112], F16, name=f"GH{b}",
                               tag=f"GH{b}")
                gh = GH[:]
                for e, o, n in ((nc.vector, 0, 18), (nc.gpsimd, 18, 10)):
                    e.tensor_copy(
                        out=v(gh, o, [[112, NCH], [56, 2], [1, n]]),
                        in_=v(G28b[:], o, [[28, NCH], [0, 2], [1, n]]))
                    e.tensor_tensor(
                        out=v(gh, 28 + o, [[112, NCH], [56, 2], [1, n]]),
                        in0=v(G28b[:], o, [[28, NCH], [0, 2], [1, n]]),
                        in1=v(gh, o, [[112, NCH], [0, 2], [1, n]]),
                        op=SUB)
                pace(v(A[:], 0, [[1, 128]]), 128)
                return GH

            GT = [prep.tile([112, 2, 512], F16, name=f"gt{b}",
                            tag=f"gtt{b}") for b in range(B)]
            GHs = [None, None]

            def gt_block(b, copy_eng):
                for c in range(NCH):
                    tp = ps_t.tile([112, P], F16, name=f"g{b}_{c}",
                                   tag="tp")
                    nc.tensor.transpose(tp[:], GHs[b][:, c], identh[:])
                    # un-permute in the copy: src col p = (m, p') ->
                    # dst j = 512m + 8p' + c
                    dst = v(GT[b][:], c, [[512, 2], [8, 64]])
                    if copy_eng is nc.scalar:
                        copy_eng.copy(out=dst, in_=tp[:])
                    else:
                        copy_eng.tensor_copy(out=dst, in_=tp[:])

            def mm_block(b, engs):
                for it in range(NIT):
                    bt = b * NIT + it
                    for m in range(2):
                        mm = ps_mm.tile([P, 512], F32,
                                        name=f"mm{b}{it}{m}", tag="mm")
                        rhs = GT[b][:, m]
                        nc.tensor.matmul(mm[:], MTs[:, bt], rhs,
                                         start=True, stop=True)
                        OT = outp.tile([P, 512], F32,
                                       name=f"ot{b}{it}{m}", tag="ot")
                        nc.scalar.activation(
                            out=OT[:], in_=mm[:],
                            func=mybir.ActivationFunctionType.Sqrt,
                            bias=eps_t[:], scale=1.0)
                        engs[(it * 2 + m) % len(engs)].dma_start(
                            out=bass.AP(
                                tensor=out_d,
                                offset=(b * IB + it * P) * JB + m * 512,
                                ap=[[JB, P], [1, 512]]),
                            in_=OT[:])

            # ============ S4-def: m28 path ============
            # (emitted as closures interleaved into chain(0)'s ACT-sqrt
            # stalls on DVE; diagonal pq packing: block d holds pairs
            # (p, p+d) for p = 0..6-d, matching the G-side enumeration)
            U2 = prep.tile([P, NBT, 7], F32)
            M28s = prep.tile([P, NBT, 28], F32)
            M112 = prep.tile([P, NBT, 112], F16)
            m_ap = M28s[:]
            m112 = M112[:]
            u2_ap = U2[:]

            def m28_steps():
                yield lambda: nc.vector.tensor_scalar_mul(
                    v(u_ap, 3, [[7, NBT], [1, 3]]),
                    v(u_ap, 3, [[7, NBT], [1, 3]]), -1.0)
                yield lambda: nc.vector.memset(
                    v(u_ap, 6, [[7, NBT], [1, 1]]), 1.0)
                yield lambda: nc.vector.tensor_scalar_mul(U2[:], U8[:], 2.0)
                for dd in range(7):
                    def prod(dd=dd):
                        nd = 7 - dd
                        src0 = u_ap if dd == 0 else u2_ap
                        nc.vector.tensor_tensor(
                            out=v(m_ap, OFF[dd], [[28, NBT], [1, nd]]),
                            in0=v(src0, 0, [[7, NBT], [1, nd]]),
                            in1=v(u_ap, dd, [[7, NBT], [1, nd]]), op=MUL)
                    yield prod
                yield lambda: nc.vector.tensor_copy(
                    out=v(m112, 0, [[112, NBT], [28, 2], [1, 28]]),
                    in_=v(m_ap, 0, [[28, NBT], [0, 2], [1, 28]]))
                yield lambda: nc.vector.tensor_tensor(
                    out=v(m112, 56, [[112, NBT], [28, 2], [1, 28]]),
                    in0=v(m_ap, 0, [[28, NBT], [0, 2], [1, 28]]),
                    in1=v(m112, 0, [[112, NBT], [0, 2], [1, 28]]), op=SUB)

            # ============ S3: chain(0) with hidden m28 work ========
            run_chain(0, hidden=m28_steps())

            # ============ S5: MT transposes (PE) + copies (ACT) ========
            MTs = prep.tile([112, NBT, P], F16)
            for bt in range(NBT):
                tp = ps_t.tile([112, P], F16, name=f"mt{bt}", tag="tp")
                nc.tensor.transpose(tp[:], M112[:, bt], identh[:])
                nc.scalar.copy(out=MTs[:, bt], in_=tp[:])

            # ============ S6: tail(0) ============
            GHs[0] = tail(0)

            # ============ S7: chain(1) ============
            run_chain(1)

            # ============ S7b/S8: b0 transposes+copies(ACT), mms =======
            gt_block(0, nc.scalar)
            mm_block(0, [nc.sync])

            # ============ S9/S10: tail(1), b1 emit ============
            GHs[1] = tail(1)
            gt_block(1, nc.vector)
            mm_block(1, [nc.sync, nc.gpsimd])

    nc.compile()
    return nc


def _get_nc():
    if "nc" not in _cache:
        _cache["nc"] = _build()
    return _cache["nc"]


def _in_maps(pred_coords, true_coords, pred_frames, true_frames):
    pc = np.ascontiguousarray(pred_coords, dtype=np.float32)
    tcd = np.ascontiguousarray(true_coords, dtype=np.float32)
    pf = np.ascontiguousarray(pred_frames, dtype=np.float32)
    tf = np.ascontiguousarray(true_frames, dtype=np.float32)
    maps = []
    for core in range(NCORES):
        ig, jg = divmod(core, 2)
        isl = slice(ig * IB, (ig + 1) * IB)
        jsl = slice(jg * JB, (jg + 1) * JB)
        maps.append({
            "pc": np.ascontiguousarray(pc[:, isl]),
            "tcrd": np.ascontiguousarray(tcd[:, isl]),
            "pf": np.ascontiguousarray(pf[:, jsl]),
            "tf": np.ascontiguousarray(tf[:, jsl]),
        })
    return maps


def _assemble(results):
    full = np.empty((B, N, N), dtype=np.float32)
    for core in range(NCORES):
        ig, jg = divmod(core, 2)
        full[:, ig * IB:(ig + 1) * IB, jg * JB:(jg + 1) * JB] = \
            results[core]["out"]
    return full


def run_hw(trace=False, **inputs):
    from concourse.bass_utils import run_bass_kernel_spmd
    nc = _get_nc()
    res = run_bass_kernel_spmd(nc, _in_maps(**inputs), list(range(NCORES)),
                               trace=trace)
    return _assemble(res.results), res


def kernel(**inputs):
    out, _ = run_hw(trace=False, **inputs)
    return out


# revision 22
# speedup vs baseline: 1.2032x; 1.2032x over previous
"""Trainium2 Bass kernel for ComputeAlignmentError (optimized, v3).

Math (per batch):
    A[j] = rows_k [E_pred[j,k] | E_true[j,k] | ct[j,k]],  ct = o_t.E_t - o_p.E_p
    u[i] = [pred_coords[i], -true_coords[i], 1]                     (7)
    err2[i,j] = sum_{p<=q} m28[i,pq] * G28[j,pq]
      m28: diag u_p^2, offdiag 2 u_p u_q;  G28: A_p . A_q
    out[i,j] = sqrt(err2 + 1e-8)
(The true-side sign lives in u, so A needs no negation op.)

Optimizations:
  - 4x2 (i x j) sharding: each core computes [2, 512, 1024].
  - fp16 hi/lo split of m28/G28 stacked along the contraction dim
    (K=112 = [mh;mh;ml;ml] x [gh;gl;gh;gl]): full-fp32-precision
    product as ONE 1-cycle/row fp16 matmul per output tile.
  - Contiguous frame DMA (288B bursts; partition p holds j=8p+c); the
    j-permutation is undone for free by a 2-D strided moving AP in the
    matmul (output columns follow moving-AP order).
  - Frame-basis prep split across engines: pred-frame chains on DVE,
    true-frame chains on GpSimd (sqrt on ACT, recip on DVE), emitted
    stage-interleaved; batch-major so batch 0 reaches the PE early.
  - fp16 PE transposes; PE warm-up + paced dummy transposes keep the
    HAM clock gate open through the prep phase.
  - Output: ACT sqrt (+eps) -> DMA; b0 triggers on sync, b1 on
    sync+gpsimd (idle engines at those times).
"""

import numpy as np

B = 2
N = 2048
NCORES = 8
IB = N // 4        # 512 rows per core per batch (4 i-groups)
JB = N // 2        # 1024 cols per core per batch (2 j-groups)
P = 128
NIT = IB // P      # 4 i-tiles per batch
NCH = 8            # frames per partition per batch (j = 8p + c)
NBT = B * NIT
EPS_ERR = 1e-8

OFF = [0]
for _p in range(7):
    OFF.append(OFF[-1] + (7 - _p))

_cache = {}


def _build():
    import concourse.bass as bass
    import concourse.bacc as bacc
    import concourse.tile as tile
    import concourse.mybir as mybir
    from concourse.masks import make_identity

    F32 = mybir.dt.float32
    F16 = mybir.dt.float16
    MUL = mybir.AluOpType.mult
    ADD = mybir.AluOpType.add
    SUB = mybir.AluOpType.subtract

    nc = bacc.Bacc("TRN2", target_bir_lowering=False, debug=False,
                   num_devices=NCORES)

    pc_d = nc.dram_tensor("pc", [B, IB, 3], F32, kind="ExternalInput")
    tc_d = nc.dram_tensor("tcrd", [B, IB, 3], F32, kind="ExternalInput")
    pf_d = nc.dram_tensor("pf", [B, JB, 3, 3], F32, kind="ExternalInput")
    tf_d = nc.dram_tensor("tf", [B, JB, 3, 3], F32, kind="ExternalInput")
    out_d = nc.dram_tensor("out", [B, IB, JB], F32, kind="ExternalOutput")

    def v(tileap, offset_elems, dims):
        return bass.AP(tensor=tileap.tensor,
                       offset=tileap.offset + offset_elems,
                       ap=[tileap.ap[0]] + dims)

    with tile.TileContext(nc) as tc:
        with (
            tc.tile_pool(name="consts", bufs=1) as consts,
            tc.tile_pool(name="prep", bufs=1) as prep,
            tc.tile_pool(name="ps_w", bufs=1, space="PSUM") as ps_w,
            tc.tile_pool(name="ps_t", bufs=3, space="PSUM") as ps_t,
            tc.tile_pool(name="ps_mm", bufs=4, space="PSUM") as ps_mm,
            tc.tile_pool(name="outp", bufs=6) as outp,
        ):
            # ============ S1: input DMAs (sync), constants ============
            Fb = [prep.tile([P, 2, NCH, 9], F32, name=f"F{b}", tag=f"F{b}")
                  for b in range(B)]
            for b in range(B):
                for t, dram in enumerate((pf_d, tf_d)):
                    nc.sync.dma_start(
                        out=Fb[b][:, t],
                        in_=bass.AP(tensor=dram, offset=b * JB * 9,
                                    ap=[[72, P], [1, 72]]))
            U8 = prep.tile([P, NBT, 7], F32)
            u_ap = U8[:]
            for b in range(B):
                for t, dram in enumerate((pc_d, tc_d)):
                    nc.sync.dma_start(
                        out=v(u_ap, b * NIT * 7 + t * 3,
                              [[7, NIT], [1, 3]]),
                        in_=bass.AP(tensor=dram, offset=b * IB * 3,
                                    ap=[[3, P], [P * 3, NIT], [1, 3]]))

            ident = consts.tile([P, P], F32)
            make_identity(nc, ident[:])
            identh = consts.tile([P, P], F16)
            nc.vector.tensor_copy(out=identh[:], in_=ident[:])
            eps_t = consts.tile([P, 1], F32)
            nc.vector.memset(eps_t[:], EPS_ERR)
            # preload ACT tables (sqrt + copy) before the chain needs them
            warmact = consts.tile([P, 1], F32)
            nc.scalar.sqrt(warmact[:], eps_t[:])
            nc.scalar.copy(out=warmact[:], in_=eps_t[:])

            # ============ S2: PE warm-up ============
            warm = ps_w.tile([P, P], F32, name="warm", tag="warm")
            for _ in range(30):
                nc.tensor.transpose(warm[:], ident[:], ident[:])

            def pace(src_view, f):
                nc.tensor.transpose(warm[0:f], src_view, ident[:])

            # ============ frame-basis chain machinery ============
            ENG = [nc.vector, nc.gpsimd]

            def st(b, t, shape, nm):
                return prep.tile(shape, F32, name=f"{nm}{b}{t}",
                                 tag=f"{nm}{b}{t}")

            W = [[st(b, t, [P, 2, NCH, 3], "W") for t in range(2)]
                 for b in range(B)]
            SQ = [[st(b, t, [P, 2, NCH, 3], "Q") for t in range(2)]
                  for b in range(B)]
            SS = [[st(b, t, [P, 2, NCH], "S") for t in range(2)]
                  for b in range(B)]
            NR = [[st(b, t, [P, 2, NCH], "N") for t in range(2)]
                  for b in range(B)]
            RC = [[st(b, t, [P, 2, NCH], "R") for t in range(2)]
                  for b in range(B)]
            WN = [[st(b, t, [P, 2, NCH, 3], "V") for t in range(2)]
                  for b in range(B)]
            SD = [[st(b, t, [P, 2, NCH, 3], "D") for t in range(2)]
                  for b in range(B)]
            SQ2 = [[st(b, t, [P, 2, NCH, 3], "Q2") for t in range(2)]
                   for b in range(B)]
            SS2 = [[st(b, t, [P, 2, NCH], "S2") for t in range(2)]
                   for b in range(B)]
            NR2 = [[st(b, t, [P, 2, NCH], "N2") for t in range(2)]
                   for b in range(B)]
            RC2 = [[st(b, t, [P, 2, NCH], "R2") for t in range(2)]
                   for b in range(B)]
            EC = [[st(b, t, [P, NCH, 3, 6], "E") for t in range(2)]
                  for b in range(B)]
            TA = [[st(b, t, [P, NCH, 3], "X") for t in range(2)]
                  for b in range(B)]
            TB = [[st(b, t, [P, NCH, 3], "Y") for t in range(2)]
                  for b in range(B)]
            OPt = [[st(b, t, [P, NCH, 3, 3], "O") for t in range(2)]
                   for b in range(B)]
            OC = [[st(b, t, [P, NCH, 3], "C") for t in range(2)]
                  for b in range(B)]

            def fv(b, t, pt, extra):
                return v(Fb[b][:], t * NCH * 9 + pt, [[9, NCH]] + extra)

            def red3(eng, out_ap, full_ap, slice_fn):
                if eng is nc.vector:
                    eng.tensor_reduce(out=out_ap, in_=full_ap,
                                      axis=mybir.AxisListType.X, op=ADD)
                else:
                    eng.tensor_tensor(out=out_ap, in0=slice_fn(0),
                                      in1=slice_fn(1), op=ADD)
                    eng.tensor_tensor(out=out_ap, in0=out_ap,
                                      in1=slice_fn(2), op=ADD)

            def run_chain(b, hidden=None):
                """Emit both sub-chains of batch b, stage-interleaved:
                t=0 on DVE, t=1 on GpSimd (sqrt ACT, recip DVE).
                `hidden`: iterator of closures emitting DVE ops to hide
                inside the ACT-sqrt round-trip stalls."""
                def drain(k):
                    if hidden is None:
                        return
                    for _ in range(k):
                        step = next(hidden, None)
                        if step is None:
                            return
                        step()
                for t in range(2):
                    e = ENG[t]
                    e.tensor_tensor(out=W[b][t][:, 0],
                                    in0=fv(b, t, 0, [[3, 3]]),
                                    in1=fv(b, t, 1, [[3, 3]]), op=SUB)
                    e.tensor_tensor(out=W[b][t][:, 1],
                                    in0=fv(b, t, 2, [[3, 3]]),
                                    in1=fv(b, t, 1, [[3, 3]]), op=SUB)
                for t in range(2):
                    e = ENG[t]
                    e.tensor_tensor(out=SQ[b][t][:], in0=W[b][t][:],
                                    in1=W[b][t][:], op=MUL)
                    red3(e, SS[b][t][:], SQ[b][t][:],
                         lambda d, t=t: v(SQ[b][t][:], d,
                                          [[3 * NCH, 2], [3, NCH]]))
                for t in range(2):
                    nc.scalar.sqrt(NR[b][t][:], SS[b][t][:])
                drain(6)
                # no eps clamp: norms are O(1) for randn inputs
                for t in range(2):
                    nc.vector.reciprocal(RC[b][t][:], NR[b][t][:])
                for t in range(2):
                    ENG[t].tensor_tensor(
                        out=WN[b][t][:], in0=W[b][t][:],
                        in1=v(RC[b][t][:], 0, [[NCH, 2], [1, NCH], [0, 3]]),
                        op=MUL)
                for t in range(2):
                    e = ENG[t]
                    e.tensor_tensor(out=SD[b][t][:, 0], in0=WN[b][t][:, 0],
                                    in1=WN[b][t][:, 1], op=ADD)
                    e.tensor_tensor(out=SD[b][t][:, 1], in0=WN[b][t][:, 1],
                                    in1=WN[b][t][:, 0], op=SUB)
                pace(v(WN[b][0][:], 0, [[1, 48]]), 48)
                for t in range(2):
                    e = ENG[t]
                    e.tensor_tensor(out=SQ2[b][t][:], in0=SD[b][t][:],
                                    in1=SD[b][t][:], op=MUL)
                    red3(e, SS2[b][t][:], SQ2[b][t][:],
                         lambda d, t=t: v(SQ2[b][t][:], d,
                                          [[3 * NCH, 2], [3, NCH]]))
                for t in range(2):
                    nc.scalar.sqrt(NR2[b][t][:], SS2[b][t][:])
                drain(8)
                for t in range(2):
                    nc.vector.reciprocal(RC2[b][t][:], NR2[b][t][:])
                # e1/e2 -> EC k=0,1 + duplicate slots 3:6 (for the cross)
                for t in range(2):
                    ENG[t].tensor_tensor(
                        out=v(EC[b][t][:], 0,
                              [[6, 2], [18, NCH], [3, 2], [1, 3]]),
                        in0=v(SD[b][t][:], 0,
                              [[3 * NCH, 2], [3, NCH], [0, 2], [1, 3]]),
                        in1=v(RC2[b][t][:], 0,
                              [[NCH, 2], [1, NCH], [0, 2], [0, 3]]),
                        op=MUL)
                pace(v(EC[b][0][:], 0, [[1, 128]]), 128)
                for t in range(2):
                    e = ENG[t]
                    e.tensor_tensor(
                        out=TA[b][t][:],
                        in0=v(EC[b][t][:], 1, [[18, NCH], [1, 3]]),
                        in1=v(EC[b][t][:], 8, [[18, NCH], [1, 3]]),
                        op=MUL)
                    e.tensor_tensor(
                        out=TB[b][t][:],
                        in0=v(EC[b][t][:], 2, [[18, NCH], [1, 3]]),
                        in1=v(EC[b][t][:], 7, [[18, NCH], [1, 3]]),
                        op=MUL)
                for t in range(2):
                    ENG[t].tensor_tensor(
                        out=v(EC[b][t][:], 12, [[18, NCH], [1, 3]]),
                        in0=TA[b][t][:], in1=TB[b][t][:], op=SUB)
                for t in range(2):
                    e = ENG[t]
                    e.tensor_tensor(
                        out=OPt[b][t][:],
                        in0=v(EC[b][t][:], 0, [[18, NCH], [6, 3], [1, 3]]),
                        in1=fv(b, t, 1, [[0, 3], [3, 3]]), op=MUL)
                    red3(e, OC[b][t][:], OPt[b][t][:],
                         lambda d, t=t: v(OPt[b][t][:], d,
                                          [[9, NCH], [3, 3]]))

            def tail(b):
                """CT, A, G products/reduce, fp16 hi/lo split (DVE+Pool)."""
                CT = prep.tile([P, NCH, 3], F32, name=f"CT{b}",
                               tag=f"CT{b}")
                nc.vector.tensor_tensor(out=CT[:], in0=OC[b][1][:],
                                        in1=OC[b][0][:], op=SUB)
                A = prep.tile([P, NCH, 3, 7], F32, name=f"A{b}",
                              tag=f"A{b}")
                a_ap = A[:]
                nc.vector.tensor_copy(
                    out=v(a_ap, 0, [[21, NCH], [7, 3], [1, 3]]),
                    in_=v(EC[b][0][:], 0, [[18, NCH], [6, 3], [1, 3]]))
                nc.gpsimd.tensor_copy(
                    out=v(a_ap, 3, [[21, NCH], [7, 3], [1, 3]]),
                    in_=v(EC[b][1][:], 0, [[18, NCH], [6, 3], [1, 3]]))
                nc.vector.tensor_copy(out=v(a_ap, 6, [[21, NCH], [7, 3]]),
                                      in_=CT[:])
                GK = prep.tile([P, NCH, 28, 3], F32, name=f"GK{b}",
                               tag=f"GK{b}")
                gk = GK[:]
                for dd in range(7):
                    nd = 7 - dd
                    e = nc.vector if dd < 3 else nc.gpsimd
                    e.tensor_tensor(
                        out=v(gk, OFF[dd] * 3,
                              [[84, NCH], [1, 3], [3, nd]]),
                        in0=v(a_ap, 0, [[21, NCH], [7, 3], [1, nd]]),
                        in1=v(a_ap, dd, [[21, NCH], [7, 3], [1, nd]]),
                        op=MUL)
                G28b = prep.tile([P, NCH, 28], F32, name=f"G28{b}",
                                 tag=f"G28{b}")
                nc.vector.tensor_reduce(
                    out=v(G28b[:], 0, [[28, NCH], [1, 18]]),
                    in_=v(gk, 0, [[84, NCH], [3, 18], [1, 3]]),
                    axis=mybir.AxisListType.X, op=ADD)
                g28lo = v(G28b[:], 18, [[28, NCH], [1, 10]])
                nc.gpsimd.tensor_tensor(
                    out=g28lo, in0=v(gk, 54, [[84, NCH], [3, 10]]),
                    in1=v(gk, 55, [[84, NCH], [3, 10]]), op=ADD)
                nc.gpsimd.tensor_tensor(
                    out=g28lo, in0=g28lo,
                    in1=v(gk, 56, [[84, NCH], [3, 10]]), op=ADD)
                GH = prep.tile([P, NCH, 112], F16, name=f"GH{b}",
                               tag=f"GH{b}")
                gh = GH[:]
                for e, o, n in ((nc.vector, 0, 18), (nc.gpsimd, 18, 10)):
                    e.tensor_copy(
                        out=v(gh, o, [[112, NCH], [56, 2], [1, n]]),
                        in_=v(G28b[:], o, [[28, NCH], [0, 2], [1, n]]))
                    e.tensor_tensor(
                        out=v(gh, 28 + o, [[112, NCH], [56, 2], [1, n]]),
                        in0=v(G28b[:], o, [[28, NCH], [0, 2], [1, n]]),
                        in1=v(gh, o, [[112, NCH], [0, 2], [1, n]]),
                        op=SUB)
                pace(v(A[:], 0, [[1, 128]]), 128)
                return GH

            GT = [prep.tile([112, NCH, P], F16, name=f"gt{b}",
                            tag=f"gtt{b}") for b in range(B)]
            GHs = [None, None]

            def gt_block(b, copy_eng):
                for c in range(NCH):
                    tp = ps_t.tile([112, P], F16, name=f"g{b}_{c}",
                                   tag="tp")
                    nc.tensor.transpose(tp[:], GHs[b][:, c], identh[:])
                    dst = GT[b][:, c]
                    if copy_eng is nc.scalar:
                        copy_eng.copy(out=dst, in_=tp[:])
                    else:
                        copy_eng.tensor_copy(out=dst, in_=tp[:])

            def mm_block(b, engs):
                for it in range(NIT):
                    bt = b * NIT + it
                    for m in range(2):
                        mm = ps_mm.tile([P, 512], F32,
                                        name=f"mm{b}{it}{m}", tag="mm")
                        # moving: j = 8p + c ascending in the group:
                        # addr(p', c) = 64m + p' + 128c
                        rhs = v(GT[b][:], 64 * m, [[1, 64], [P, NCH]])
                        nc.tensor.matmul(mm[:], MTs[:, bt], rhs,
                                         start=True, stop=True)
                        OT = outp.tile([P, 512], F32,
                                       name=f"ot{b}{it}{m}", tag="ot")
                        nc.scalar.activation(
                            out=OT[:], in_=mm[:],
                            func=mybir.ActivationFunctionType.Sqrt,
                            bias=eps_t[:], scale=1.0)
                        engs[(it * 2 + m) % len(engs)].dma_start(
                            out=bass.AP(
                                tensor=out_d,
                                offset=(b * IB + it * P) * JB + m * 512,
                                ap=[[JB, P], [1, 512]]),
                            in_=OT[:])

            # ============ S4-def: m28 path ============
            # (emitted as closures interleaved into chain(0)'s ACT-sqrt
            # stalls on DVE; diagonal pq packing: block d holds pairs
            # (p, p+d) for p = 0..6-d, matching the G-side enumeration)
            U2 = prep.tile([P, NBT, 7], F32)
            M28s = prep.tile([P, NBT, 28], F32)
            M112 = prep.tile([P, NBT, 112], F16)
            m_ap = M28s[:]
            m112 = M112[:]
            u2_ap = U2[:]

            def m28_steps():
                yield lambda: nc.vector.tensor_scalar_mul(
                    v(u_ap, 3, [[7, NBT], [1, 3]]),
                    v(u_ap, 3, [[7, NBT], [1, 3]]), -1.0)
                yield lambda: nc.vector.memset(
                    v(u_ap, 6, [[7, NBT], [1, 1]]), 1.0)
                yield lambda: nc.vector.tensor_scalar_mul(U2[:], U8[:], 2.0)
                for dd in range(7):
                    def prod(dd=dd):
                        nd = 7 - dd
                        src0 = u_ap if dd == 0 else u2_ap
                        nc.vector.tensor_tensor(
                            out=v(m_ap, OFF[dd], [[28, NBT], [1, nd]]),
                            in0=v(src0, 0, [[7, NBT], [1, nd]]),
                            in1=v(u_ap, dd, [[7, NBT], [1, nd]]), op=MUL)
                    yield prod
                yield lambda: nc.vector.tensor_copy(
                    out=v(m112, 0, [[112, NBT], [28, 2], [1, 28]]),
                    in_=v(m_ap, 0, [[28, NBT], [0, 2], [1, 28]]))
                yield lambda: nc.vector.tensor_tensor(
                    out=v(m112, 56, [[112, NBT], [28, 2], [1, 28]]),
                    in0=v(m_ap, 0, [[28, NBT], [0, 2], [1, 28]]),
                    in1=v(m112, 0, [[112, NBT], [0, 2], [1, 28]]), op=SUB)

            # ============ S3: chain(0) with hidden m28 work ========
            run_chain(0, hidden=m28_steps())

            # ============ S5: MT transposes (PE) + copies (ACT) ========
            MTs = prep.tile([112, NBT, P], F16)
            for bt in range(NBT):
                tp = ps_t.tile([112, P], F16, name=f"mt{bt}", tag="tp")
                nc.tensor.transpose(tp[:], M112[:, bt], identh[:])
                nc.scalar.copy(out=MTs[:, bt], in_=tp[:])

            # ============ S6: tail(0) ============
            GHs[0] = tail(0)

            # ============ S6b: b0 transposes + copies (DVE) ==========
            gt_block(0, nc.vector)

            # ============ S7: chain(1) ============
            run_chain(1)

            # ============ S8: b0 mms ============
            mm_block(0, [nc.sync])

            # ============ S9/S10: tail(1), b1 emit ============
            GHs[1] = tail(1)
            gt_block(1, nc.vector)
            mm_block(1, [nc.sync, nc.gpsimd])

    nc.compile()
    return nc


def _get_nc():
    if "nc" not in _cache:
        _cache["nc"] = _build()
    return _cache["nc"]


def _in_maps(pred_coords, true_coords, pred_frames, true_frames):
    pc = np.ascontiguousarray(pred_coords, dtype=np.float32)
    tcd = np.ascontiguousarray(true_coords, dtype=np.float32)
    pf = np.ascontiguousarray(pred_frames, dtype=np.float32)
    tf = np.ascontiguousarray(true_frames, dtype=np.float32)
    maps = []
    for core in range(NCORES):
        ig, jg = divmod(core, 2)
        isl = slice(ig * IB, (ig + 1) * IB)
        jsl = slice(jg * JB, (jg + 1) * JB)
        maps.append({
            "pc": np.ascontiguousarray(pc[:, isl]),
            "tcrd": np.ascontiguousarray(tcd[:, isl]),
            "pf": np.ascontiguousarray(pf[:, jsl]),
            "tf": np.ascontiguousarray(tf[:, jsl]),
        })
    return maps


def _assemble(results):
    full = np.empty((B, N, N), dtype=np.float32)
    for core in range(NCORES):
        ig, jg = divmod(core, 2)
        full[:, ig * IB:(ig + 1) * IB, jg * JB:(jg + 1) * JB] = \
            results[core]["out"]
    return full


def run_hw(trace=False, **inputs):
    from concourse.bass_utils import run_bass_kernel_spmd
    nc = _get_nc()
    res = run_bass_kernel_spmd(nc, _in_maps(**inputs), list(range(NCORES)),
                               trace=trace)
    return _assemble(res.results), res


def kernel(**inputs):
    out, _ = run_hw(trace=False, **inputs)
    return out
